# revision 42
# baseline (speedup 1.0000x reference)
"""GCN encoder layer (degree-normalized message passing + BN inference) on 8 Trainium2 cores.

Math (see reference):
    t = X @ W + b                                  [N, H]
    deg = out-degree by src                        [N]
    isd = deg ** -0.5
    nb_sum[i]  = isd[i] * sum_{e: src=i} isd[dst_e] * t[dst_e]
    src_mean   = deg * t            (segment_mean(deg[src]*t[src]) simplifies exactly)
    agg = 0.5*nb_sum + 0.5*src_mean
    out = (agg - mean) * rsqrt(var+eps) * gamma + beta

Strategy (edge-parallel, sharded by src range -> no cross-core collective):
  - Core c owns src nodes [c*6250, (c+1)*6250); its edges are grouped into
    49 windows of 128 local segments, sorted by dst within a window.
  - Gather X[dst] rows as bf16 (256B packets) from HBM via gpsimd.dma_gather.
    Indices are int16, so the node table is addressed as two views
    (dst < 32768 and dst >= 32768); each window's edges form a low run then
    a high run, each padded to a multiple of 128 ("batches").
  - Scatter-add via one-hot matmuls on the PE:  Z_T[f, s] += G.T @ O  where
    G = gathered X rows [128 edges, 128 feat] bf16 (stationary) and
    O[e, s] = (s == src_local[e]) * 0.5*isd[src_e]*isd[dst_e].  O is
    HOST-PRECOMPUTED in fp8e4m3 and DMA'd in (cheaper than building the
    one-hot on DVE, whose broadcast APs run at 1 elem/cycle).
  - Aggregation commutes with @W:  nb_T = W16.T @ Z16  accumulated in PSUM
    together with the source term  W.T @ (0.5*deg*X_own)_T  (host-prepared
    pre-scaled AND pre-transposed) and a rank-1 bias term when b != 0.
  - Windows are processed in groups of 4 (512-wide PSUM tiles) to amortize
    stage-2 matmuls / BN; output written feature-major, transposed on host.
"""

import math
import numpy as np
import ml_dtypes

N_CORES = 8
P = 128
F = 128
H = 128
BN_EPS = 1e-3
SPLIT = 32768      # int16 index limit for dma_gather
CHB = 32           # gather chunk size in batches
OCH = 16           # one-hot (O) load chunk size in batches
GW = 2             # windows per stage-2 group (256-wide PSUM)

_CACHE = {}


def _wrap16(arr):
    """dma_gather index layout: unwrapped[i] = w[i%16, i//16], replicated x8."""
    w = arr.reshape(-1, 16).T.copy()
    return np.ascontiguousarray(np.tile(w, (8, 1)))


def _build_host_data(edge_pairs, node_features):
    n_nodes = node_features.shape[0]
    src = np.asarray(edge_pairs[:, 0], dtype=np.int64)
    dst = np.asarray(edge_pairs[:, 1], dtype=np.int64)
    deg = np.bincount(src, minlength=n_nodes).astype(np.float64)

    npc = n_nodes // N_CORES
    assert npc * N_CORES == n_nodes
    NW = math.ceil(npc / P)
    NG = math.ceil(NW / GW)
    npc_pad = NG * GW * P

    core = src // npc
    win = (src - core * npc) // P
    srcl = (src - core * npc) % P
    half = (dst >= SPLIT).astype(np.int64)

    order = np.lexsort((dst, half, win, core))
    dst_s = dst[order]
    core_s, win_s, srcl_s, half_s = core[order], win[order], srcl[order], half[order]
    with np.errstate(divide="ignore"):
        scl_s = (0.5 / np.sqrt(deg[src[order]] * deg[dst_s])).astype(np.float32)

    # counts per (core, window, half); batch counts shared across cores (SPMD)
    cnt = np.zeros((N_CORES, NW, 2), dtype=np.int64)
    np.add.at(cnt, (core_s, win_s, half_s), 1)
    nbL = np.ceil(cnt[:, :, 0].max(axis=0) / P).astype(np.int64)  # [NW]
    nbH = np.ceil(cnt[:, :, 1].max(axis=0) / P).astype(np.int64)
    NBL, NBH = int(nbL.sum()), int(nbH.sum())
    NBtot = NBL + NBH
    cumL = np.concatenate([[0], np.cumsum(nbL)])   # stream-L batch base per window
    cumH = np.concatenate([[0], np.cumsum(nbH)])

    # run starts in the sorted edge array per (core, window, half)
    flat = cnt.reshape(-1)
    starts_flat = np.concatenate([[0], np.cumsum(flat)[:-1]])
    starts = starts_flat.reshape(N_CORES, NW, 2)

    IDXL = np.zeros((N_CORES, NBL * P), dtype=np.int16)
    IDXH = np.zeros((N_CORES, NBH * P), dtype=np.int16)
    # host-precomputed scaled one-hot, laid out [partition(edge slot), cb*128+s]
    OD = np.zeros((N_CORES, P, NBtot * P), dtype=ml_dtypes.float8_e4m3)
    # per-batch metadata for the on-chip (DVE) one-hot build
    SRCL16 = np.full((N_CORES, P, NBtot), -1.0, dtype=np.float32)
    SCL16 = np.ones((N_CORES, P, NBtot), dtype=np.float32)

    for c in range(N_CORES):
        for w in range(NW):
            for h, (nb_arr, cum, IDX, off) in enumerate(
                    ((nbL, cumL, IDXL, 0), (nbH, cumH, IDXH, SPLIT))):
                nbw = int(nb_arr[w])
                if nbw == 0:
                    continue
                a = starts[c, w, h]
                n = int(cnt[c, w, h])
                nslots = nbw * P
                d_pad = np.zeros(nslots, dtype=np.int16)
                if n > 0:
                    d_pad[:n] = (dst_s[a:a + n] - off).astype(np.int16)
                    d_pad[n:] = d_pad[n - 1]
                sb = int(cum[w])           # stream batch base
                IDX[c, sb * P:(sb + nbw) * P] = d_pad
                # combined batch index: L block then H block
                cb = sb + (NBL if h == 1 else 0)
                if n > 0:
                    k = np.arange(n) // P          # batch within run
                    p = np.arange(n) % P           # partition (edge slot)
                    cols = (cb + k) * P + srcl_s[a:a + n]
                    OD[c, p, cols] = scl_s[a:a + n].astype(ml_dtypes.float8_e4m3)
                    SRCL16[c, p, cb + k] = srcl_s[a:a + n]
                    SCL16[c, p, cb + k] = scl_s[a:a + n]

    # gather-chunk boundaries (even split; shared device/host) and the
    # O routing: first half of each chunk's batches comes from HBM (fp8),
    # second half is built on the DVE. OD is compacted to the HBM halves.
    bounds = {}
    for sname, nb_s in (("L", NBL), ("H", NBH)):
        nch = max(1, math.ceil(nb_s / CHB))
        bounds[sname] = [round(i * nb_s / nch) for i in range(nch + 1)]
    osub = {}          # (stream, chunk, sub) -> (cb0, cb1, kind, od_off)
    od_cols = 0
    for sname, nb_s, base in (("L", NBL, 0), ("H", NBH, NBL)):
        bnds = bounds[sname]
        for ci in range(len(bnds) - 1):
            c0, c1 = bnds[ci], bnds[ci + 1]
            mid = (c0 + c1 + 1) // 2
            osub[(sname, ci, 0)] = (base + c0, base + mid, "hbm", od_cols)
            od_cols += (mid - c0) * P
            osub[(sname, ci, 1)] = (base + mid, base + c1, "hbm", od_cols)
            od_cols += (c1 - mid) * P
    ODC = np.zeros((N_CORES, P, max(od_cols, P)), dtype=ml_dtypes.float8_e4m3)
    for (sname, ci, sub), (cb0, cb1, kind, off) in osub.items():
        if kind == "hbm" and cb1 > cb0:
            ODC[:, :, off:off + (cb1 - cb0) * P] = OD[:, :, cb0 * P:cb1 * P]
    IOTA16 = np.tile(np.arange(P, dtype=np.float32), (P, 1)).astype(ml_dtypes.bfloat16)

    nf = np.asarray(node_features, dtype=np.float32)
    NF16 = nf.astype(ml_dtypes.bfloat16)

    XOT = np.zeros((N_CORES, P, npc_pad), dtype=np.float32)
    BCOEF = np.zeros((N_CORES, 1, npc_pad), dtype=np.float32)
    isd = np.where(deg > 0, deg ** -0.5, np.inf)
    nb_isd_sum = np.zeros(n_nodes, dtype=np.float64)
    np.add.at(nb_isd_sum, src, np.where(deg[dst] > 0, deg[dst] ** -0.5, np.inf))
    bcoef_full = 0.5 * deg + 0.5 * isd * nb_isd_sum
    bcoef_full = np.where(deg > 0, bcoef_full, 0.0)
    for c in range(N_CORES):
        xo = nf[c * npc:(c + 1) * npc] * (0.5 * deg[c * npc:(c + 1) * npc])[:, None]
        XOT[c, :, :npc] = xo.T.astype(np.float32)
        BCOEF[c, 0, :npc] = bcoef_full[c * npc:(c + 1) * npc].astype(np.float32)

    IDXLw = np.stack([_wrap16(IDXL[c]) for c in range(N_CORES)]) if NBL else \
        np.zeros((N_CORES, P, 0), np.int16)
    IDXHw = np.stack([_wrap16(IDXH[c]) for c in range(N_CORES)]) if NBH else \
        np.zeros((N_CORES, P, 0), np.int16)

    return dict(IDXL=IDXLw, IDXH=IDXHw, ODC=ODC, SRCL16=SRCL16, SCL16=SCL16,
                IOTA16=IOTA16, NF16=NF16, XOT=XOT, BCOEF=BCOEF,
                NW=NW, NG=NG, NBL=NBL, NBH=NBH, NBtot=NBtot,
                nbL=nbL, nbH=nbH, cumL=cumL, cumH=cumH,
                bounds=bounds, osub=osub, od_cols=od_cols,
                npc=npc, npc_pad=npc_pad)


def _build_nc(hd, n_nodes, has_b):
    import concourse.bass as bass
    import concourse.bacc as bacc
    import concourse.mybir as mybir
    import concourse.tile as tile

    NW, NG = hd["NW"], hd["NG"]
    NBL, NBH, NBtot = hd["NBL"], hd["NBH"], hd["NBtot"]
    nbL, nbH = hd["nbL"], hd["nbH"]
    cumL, cumH = hd["cumL"], hd["cumH"]
    npc_pad = hd["npc_pad"]

    fp32 = mybir.dt.float32
    bf16 = mybir.dt.bfloat16
    fp8 = mybir.dt.float8e4
    nc = bacc.Bacc("TRN2", target_bir_lowering=False, debug=False,
                   num_swdge_queues=4)

    od_cols = hd["od_cols"]
    nf_d = nc.dram_tensor("NF16", [n_nodes, F], bf16, kind="ExternalInput")
    il_d = nc.dram_tensor("IDXL", [P, max(NBL, 1) * 8], mybir.dt.int16, kind="ExternalInput")
    ih_d = nc.dram_tensor("IDXH", [P, max(NBH, 1) * 8], mybir.dt.int16, kind="ExternalInput")
    od_d = nc.dram_tensor("ODC", [P, max(od_cols, P)], fp8, kind="ExternalInput")
    srcl_d = nc.dram_tensor("SRCL16", [P, NBtot], fp32, kind="ExternalInput")
    scl_d = nc.dram_tensor("SCL16", [P, NBtot], fp32, kind="ExternalInput")
    iota_d = nc.dram_tensor("IOTA16", [P, P], bf16, kind="ExternalInput")
    xot_d = nc.dram_tensor("XOT", [P, npc_pad], fp32, kind="ExternalInput")
    w_d = nc.dram_tensor("WM", [F, H], fp32, kind="ExternalInput")
    w16_d = nc.dram_tensor("WM16", [F, H], bf16, kind="ExternalInput")
    gm_d = nc.dram_tensor("GCOL", [P, 1], fp32, kind="ExternalInput")
    bt_d = nc.dram_tensor("BTCOL", [P, 1], fp32, kind="ExternalInput")
    mm_d = nc.dram_tensor("MMCOL", [P, 1], fp32, kind="ExternalInput")
    mv_d = nc.dram_tensor("MVCOL", [P, 1], fp32, kind="ExternalInput")
    if has_b:
        brow_d = nc.dram_tensor("BROW", [1, H], fp32, kind="ExternalInput")
        bcoef_d = nc.dram_tensor("BCOEF", [1, npc_pad], fp32, kind="ExternalInput")
    out_d = nc.dram_tensor("OUT_T", [P, npc_pad], fp32, kind="ExternalOutput")

    with tile.TileContext(nc) as tc:
        with (
            tc.tile_pool(name="meta", bufs=1) as meta,
            tc.tile_pool(name="ix", bufs=4) as ixpool,
            tc.tile_pool(name="gl", bufs=5) as glpool,
            tc.tile_pool(name="gh", bufs=4) as ghpool,
            tc.tile_pool(name="od", bufs=3) as odpool,
            tc.tile_pool(name="xot", bufs=2) as xotpool,
            tc.tile_pool(name="z16", bufs=2) as z16pool,
            tc.tile_pool(name="ob", bufs=2) as obpool,
            tc.tile_pool(name="psz", bufs=2, space="PSUM") as psZ,
            tc.tile_pool(name="psnb", bufs=2, space="PSUM") as psNB,
        ):
            # tiny warm-up gather: absorbs the SWDGE cold-start (Q7 library
            # load) while the metadata DMAs run
            dum_ix = meta.tile([P, 1], mybir.dt.int16)
            nc.vector.memset(dum_ix[:], 0)
            dum_g = meta.tile([P, 1, F], bf16)
            nc.gpsimd.dma_gather(dum_g[:], nf_d[0:SPLIT], dum_ix[:],
                                 16, 16, F, single_packet=False,
                                 queue_num=0)

            w_sb = meta.tile([F, H], fp32)
            w16_sb = meta.tile([F, H], bf16)
            srcl_sb = meta.tile([P, NBtot], fp32)
            scl_sb = meta.tile([P, NBtot], fp32)
            iota_sb = meta.tile([P, P], bf16)
            gm_sb = meta.tile([P, 1], fp32)
            bt_sb = meta.tile([P, 1], fp32)
            mm_sb = meta.tile([P, 1], fp32)
            mv_sb = meta.tile([P, 1], fp32)
            rs_sb = meta.tile([P, 1], fp32)
            gp_sb = meta.tile([P, 1], fp32)
            bb_sb = meta.tile([P, 1], fp32)

            nc.sync.dma_start(w_sb[:], w_d[:])
            nc.sync.dma_start(w16_sb[:], w16_d[:])
            nc.sync.dma_start(srcl_sb[:], srcl_d[:])
            nc.sync.dma_start(scl_sb[:], scl_d[:])
            nc.sync.dma_start(iota_sb[:], iota_d[:])
            nc.sync.dma_start(gm_sb[:], gm_d[:])
            nc.sync.dma_start(bt_sb[:], bt_d[:])
            nc.sync.dma_start(mm_sb[:], mm_d[:])
            nc.sync.dma_start(mv_sb[:], mv_d[:])

            # BN: g' = gamma * rsqrt(var+eps);  bb = beta - mean*g'
            nc.vector.tensor_scalar(out=rs_sb[:], in0=mv_sb[:], scalar1=BN_EPS,
                                    scalar2=None, op0=mybir.AluOpType.add)
            nc.scalar.activation(rs_sb[:], rs_sb[:], mybir.ActivationFunctionType.Sqrt)
            nc.vector.reciprocal(rs_sb[:], rs_sb[:])
            nc.vector.tensor_tensor(out=gp_sb[:], in0=gm_sb[:], in1=rs_sb[:],
                                    op=mybir.AluOpType.mult)
            nc.vector.tensor_tensor(out=bb_sb[:], in0=mm_sb[:], in1=gp_sb[:],
                                    op=mybir.AluOpType.mult)
            nc.vector.tensor_tensor(out=bb_sb[:], in0=bt_sb[:], in1=bb_sb[:],
                                    op=mybir.AluOpType.subtract)

            if has_b:
                brow_sb = meta.tile([1, H], fp32)
                bcoef_sb = meta.tile([1, npc_pad], fp32)
                nc.sync.dma_start(brow_sb[:], brow_d[:])
                nc.sync.dma_start(bcoef_sb[:], bcoef_d[:])

            # ---- gather machinery: two streams (low/high table halves) ----
            streams = {
                "L": dict(nb=NBL, idxd=il_d, view=nf_d[0:min(SPLIT, n_nodes)],
                          pool=glpool, tiles={}),
                "H": dict(nb=NBH, idxd=ih_d, view=(nf_d[SPLIT:n_nodes]
                                                   if n_nodes > SPLIT else None),
                          pool=ghpool, tiles={}),
            }

            # batch consumption order (stream, stream-batch-index)
            use_order = []
            for w in range(NW):
                for k in range(int(nbL[w])):
                    use_order.append(("L", int(cumL[w]) + k))
                for k in range(int(nbH[w])):
                    use_order.append(("H", int(cumH[w]) + k))

            # even chunk boundaries per stream (balanced SWDGE queue loads)
            bounds = hd["bounds"]
            for sname, st in streams.items():
                st["chunk_of"] = np.searchsorted(bounds[sname],
                                                 np.arange(st["nb"]),
                                                 side="right") - 1

            # emit all gathers up front, in consumption (first-use) order
            chunk_order, seen = [], set()
            for sname, sj in use_order:
                key = (sname, int(streams[sname]["chunk_of"][sj]))
                if key not in seen:
                    seen.add(key)
                    chunk_order.append(key)
            for sname, ci in chunk_order:
                st = streams[sname]
                c0, c1 = bounds[sname][ci], bounds[sname][ci + 1]
                nbc = c1 - c0
                # chunked idx load: gather #i only waits for its own slice
                ixt = ixpool.tile([P, nbc * 8], mybir.dt.int16, tag="ix" + sname)
                nc.sync.dma_start(ixt[:], st["idxd"][:, c0 * 8:c1 * 8])
                gt = st["pool"].tile([P, nbc, F], bf16, tag="g" + sname)
                nidx = nbc * P
                nc.gpsimd.dma_gather(
                    gt[:], st["view"], ixt[:],
                    nidx, nidx, F, single_packet=False, queue_num=0)
                st["tiles"][ci] = (c0, gt)

            def gslice(s, j):
                c0, gt = streams[s]["tiles"][int(streams[s]["chunk_of"][j])]
                return gt[:, j - c0, :]

            # ---- one-hot (O) tiles: first half of each gather chunk comes
            # from HBM (fp8, host-precomputed), second half is built on the
            # otherwise-idle DVE: (iota == srcl) * scl, one fused
            # tensor_scalar per batch.  This halves both the input upload
            # and the bursty 2KB-packet HBM traffic that stalls the
            # gather queues.
            osub = hd["osub"]
            otiles = {}
            sub_of = {}
            for (sname, ci, sub), (cb0, cb1, kind, off) in osub.items():
                for cb in range(cb0, cb1):
                    sub_of[cb] = (sname, ci, sub)
            oorder, oseen = [], set()
            for sname, sj in use_order:
                cb = sj + (NBL if sname == "H" else 0)
                key = sub_of[cb]
                if key not in oseen:
                    oseen.add(key)
                    oorder.append(key)
            for key in oorder:
                cb0, cb1, kind, off = osub[key]
                nbo = cb1 - cb0
                if nbo <= 0:
                    continue
                ot = odpool.tile([P, nbo * P], fp8, tag="o8")
                nc.sync.dma_start(ot[:], od_d[:, off:off + nbo * P])
                otiles[key] = (cb0, ot)

            def oslice(sname, sj):
                cb = sj + (NBL if sname == "H" else 0)
                cb0, ot = otiles[sub_of[cb]]
                return ot[:, (cb - cb0) * P:(cb - cb0 + 1) * P]

            # ---- main loop: groups of GW windows, 512-wide stage 2 ----
            for g in range(NG):
                w0 = g * GW
                nwin = min(GW, NW - w0)
                wg = nwin * P

                psz = psZ.tile([P, GW * P], fp32)
                for j in range(nwin):
                    w = w0 + j
                    nl, nh = int(nbL[w]), int(nbH[w])
                    nbw = nl + nh
                    sl = psz[:, j * P:(j + 1) * P]
                    assert nbw > 0, f"window {w} has no edge batches"
                    for k in range(nbw):
                        if k < nl:
                            sname, sj = "L", int(cumL[w]) + k
                        else:
                            sname, sj = "H", int(cumH[w]) + (k - nl)
                        nc.tensor.matmul(sl, lhsT=gslice(sname, sj),
                                         rhs=oslice(sname, sj),
                                         start=(k == 0), stop=(k == nbw - 1))

                z16 = z16pool.tile([P, GW * P], bf16)
                nc.scalar.activation(z16[:, :wg], psz[:, :wg],
                                     mybir.ActivationFunctionType.Identity)

                xot_t = xotpool.tile([P, GW * P], fp32)
                nc.sync.dma_start(xot_t[:, :wg], xot_d[:, w0 * P:w0 * P + wg])

                psnb = psNB.tile([P, GW * P], fp32)
                nc.tensor.matmul(psnb[:, :wg], lhsT=w16_sb[:], rhs=z16[:, :wg],
                                 start=True, stop=False)
                nc.tensor.matmul(psnb[:, :wg], lhsT=w_sb[:], rhs=xot_t[:, :wg],
                                 start=False, stop=not has_b)
                if has_b:
                    nc.tensor.matmul(psnb[:, :wg], lhsT=brow_sb[:],
                                     rhs=bcoef_sb[:, w0 * P:w0 * P + wg],
                                     start=False, stop=True)

                # BN affine (per-partition in feature-major layout)
                ob = obpool.tile([P, GW * P], fp32)
                nc.scalar.activation(
                    ob[:, :wg], psnb[:, :wg],
                    mybir.ActivationFunctionType.Identity,
                    bias=bb_sb[:], scale=gp_sb[:],
                )
                nc.sync.dma_start(out_d[:, w0 * P:w0 * P + wg], ob[:, :wg])

    # SWDGE queue ownership: each DMASW sem lane is owned by one queue.
    # Balance queues greedily by cumulative packet count (queue drain at
    # ~8.3ns/packet is the wall, so imbalance = wasted wall-clock), while
    # keeping the lane -> queue map consistent per proc.
    from concourse.tile_scheduler import PROC_NAME_TO_IDX
    import concourse.mybir as mybir
    idx_to_proc = {v: k for k, v in PROC_NAME_TO_IDX.items()}
    qload = [0, 0, 0, 0]
    proc_q = {}
    for bb in nc.main_func.blocks:
        for ins in bb.instructions:
            if isinstance(ins, mybir.InstDMAGatherAnt):
                proc = idx_to_proc.get(ins.bass_scheduled_proc, "")
                if proc.startswith("DMASW"):
                    if proc not in proc_q:
                        proc_q[proc] = int(proc[5:]) % 4
                    q = proc_q[proc]
                    ins.queue_num = q
                    qload[q] += ins.num_idxs

    nc.compile()
    return nc


def _prepare(edge_pairs, node_features, W, b, gamma, beta, moving_mean, moving_var):
    n_nodes, _ = node_features.shape
    hd = _build_host_data(edge_pairs, node_features)
    has_b = bool(np.any(np.asarray(b) != 0))

    key = (n_nodes, node_features.shape[1], hd["NBtot"],
           tuple(hd["nbL"].tolist()), tuple(hd["nbH"].tolist()), has_b)
    if key not in _CACHE:
        _CACHE.clear()
        _CACHE[key] = _build_nc(hd, n_nodes, has_b)
    nc = _CACHE[key]

    w32 = np.ascontiguousarray(np.asarray(W, dtype=np.float32))
    in_maps = []
    for c in range(N_CORES):
        m = {
            "NF16": hd["NF16"],
            "IDXL": np.ascontiguousarray(hd["IDXL"][c]) if hd["NBL"] else
                np.zeros((P, 8), np.int16),
            "IDXH": np.ascontiguousarray(hd["IDXH"][c]) if hd["NBH"] else
                np.zeros((P, 8), np.int16),
            "ODC": np.ascontiguousarray(hd["ODC"][c]),
            "SRCL16": np.ascontiguousarray(hd["SRCL16"][c]),
            "SCL16": np.ascontiguousarray(hd["SCL16"][c]),
            "IOTA16": hd["IOTA16"],
            "XOT": np.ascontiguousarray(hd["XOT"][c]),
            "WM": w32,
            "WM16": np.ascontiguousarray(w32.astype(ml_dtypes.bfloat16)),
            "GCOL": np.asarray(gamma, np.float32).reshape(P, 1).copy(),
            "BTCOL": np.asarray(beta, np.float32).reshape(P, 1).copy(),
            "MMCOL": np.asarray(moving_mean, np.float32).reshape(P, 1).copy(),
            "MVCOL": np.asarray(moving_var, np.float32).reshape(P, 1).copy(),
        }
        if has_b:
            m["BROW"] = np.asarray(b, np.float32).reshape(1, H).copy()
            m["BCOEF"] = np.ascontiguousarray(hd["BCOEF"][c])
        in_maps.append(m)
    return nc, in_maps, hd


def _run(inputs, trace=False):
    from concourse.bass_utils import run_bass_kernel_spmd

    nc, in_maps, hd = _prepare(**inputs)
    res = run_bass_kernel_spmd(nc, in_maps, core_ids=list(range(N_CORES)),
                               trace=trace)
    npc = hd["npc"]
    out = np.empty((npc * N_CORES, H), dtype=np.float32)
    for c in range(N_CORES):
        out[c * npc:(c + 1) * npc] = res.results[c]["OUT_T"].T[:npc]
    return out, res


def kernel(**inputs):
    out, _ = _run(inputs, trace=False)
    return out


def run_traced(**inputs):
    return _run(inputs, trace=True)
